# revision 23
# baseline (speedup 1.0000x reference)
"""Multi-head causal attention (QKV proj + RoPE + softmax) on 8 TRN2 NeuronCores.

Sharding: batch 4-way x head-group 2-way -> each core handles 1 batch and 8
contiguous heads (512 output channels). No collectives; host gathers slices.

Per-core algorithm (all matmul compute in bf16, fp32 PSUM accumulation):
  - host passes x.T (q/k/v of its batch, transposed to [emb, seq]) and W.T
    shards so every matmul contracts over the partition dim without on-device
    transposes.
  - q/k weights are row-permuted per head into [even dims | odd dims] so RoPE
    becomes: rot = x*cs + swap32(x)*sn, where swap32 is an SBUF partition-block
    swap done by DMA. The per-head dim permutation cancels in q.k dot products.
  - q/k biases are per-partition columns folded into the PSUM eviction
    (tensor_scalar add); the v bias is applied on host: P@(V+b) = P@V + l*b.
  - scores are computed transposed, S_T[k, q] = kh_T.T @ qh_T (K=64
    contraction; the A/B heads of a 128-row tile are emitted adjacently so
    they run concurrently on PE row groups).
  - softmax: exp on ScalarE from PSUM at [128,1024] granularity (no max
    subtraction: |scores| <= ~5 by construction), causal mask multiplies on
    GpSimd for the diagonal pairs only; fully-masked k-tiles are skipped.
  - attnT[d, q] = sum_kt V_tile[k,d|1].T @ P_T[k, q] -- a ones-column appended
    to V makes row 64 the softmax denominator for free.
  - unnormalized attnT and the denominator row go to HBM via one SBUF staging
    copy; division + final transpose + v-bias happen on host.
"""

import sys
import types

import numpy as np
import ml_dtypes

BF16 = ml_dtypes.bfloat16
SEQ, EMB, NHEADS, BATCH = 2048, 1024, 16, 4
HD, HALF = 64, 32
HPC = 8          # heads per core
DH = 512         # output dims per core
NE = EMB // 128  # 8 contraction tiles
NT = 4           # head-pair (128-row) dout tiles
NKT = SEQ // 128  # 16 key tiles
NQC = SEQ // 512  # 4 query chunks


def _install_ntff_shim():
    """The image's antenv lacks axon_hooks; synthesize it from trn_agent_boot
    so run_bass_kernel_spmd(trace=True) can profile. Harmless if unused."""
    try:
        import antenv.axon_hooks  # noqa: F401
        return
    except ImportError:
        pass
    try:
        from trn_agent_boot.trn_boot import _ntff_profile_via_ctypes
        import antenv
    except ImportError:
        return
    hook = _ntff_profile_via_ctypes("/opt/axon/libaxon_pjrt.so")
    mod = types.ModuleType("antenv.axon_hooks")
    mod.get_axon_ntff_profile_hook = lambda: hook
    mod.set_axon_ntff_profile_hook = lambda h: None
    sys.modules["antenv.axon_hooks"] = mod
    antenv.axon_hooks = mod


_built = {}


def build(causal=True):
    if causal in _built:
        return _built[causal]
    import concourse.mybir as mybir
    import concourse.tile as tile
    from concourse import bacc

    f32 = mybir.dt.float32
    bf = mybir.dt.bfloat16
    EXP = mybir.ActivationFunctionType.Exp
    MUL = mybir.AluOpType.mult
    ADD = mybir.AluOpType.add

    nc = bacc.Bacc(None, target_bir_lowering=False, debug=False)
    with tile.TileContext(nc) as tc:
        with tc.tile_pool(name="dram", bufs=1, space="DRAM") as dram:
            xq_d = dram.tile([EMB, SEQ], bf, kind="ExternalInput", name="xq", uniquify=False)
            xk_d = dram.tile([EMB, SEQ], bf, kind="ExternalInput", name="xk", uniquify=False)
            xv_d = dram.tile([EMB, SEQ], bf, kind="ExternalInput", name="xv", uniquify=False)
            wq_d = dram.tile([EMB, DH], bf, kind="ExternalInput", name="wq", uniquify=False)
            wk_d = dram.tile([EMB, DH], bf, kind="ExternalInput", name="wk", uniquify=False)
            wv_d = dram.tile([EMB, DH], bf, kind="ExternalInput", name="wv", uniquify=False)
            bqc_d = dram.tile([128, NT], f32, kind="ExternalInput", name="bqc", uniquify=False)
            bkc_d = dram.tile([128, NT], f32, kind="ExternalInput", name="bkc", uniquify=False)
            cs_d = dram.tile([128, SEQ], bf, kind="ExternalInput", name="cs2", uniquify=False)
            sn_d = dram.tile([128, SEQ], bf, kind="ExternalInput", name="sn2", uniquify=False)
            mk_d = dram.tile([128, 128], bf, kind="ExternalInput", name="msk", uniquify=False)
            pw_d = dram.tile([128, 128], bf, kind="ExternalInput", name="psw", uniquify=False)
            outT_d = dram.tile([DH, SEQ], f32, kind="ExternalOutput", name="outT", uniquify=False)
            l_d = dram.tile([HPC, SEQ], f32, kind="ExternalOutput", name="lsum", uniquify=False)

            with tc.tile_pool(name="const", bufs=1) as cp, \
                 tc.tile_pool(name="xin", bufs=10) as xp, \
                 tc.tile_pool(name="rope", bufs=2) as rp, \
                 tc.tile_pool(name="ostage", bufs=2) as op, \
                 tc.tile_pool(name="pp", bufs=2, space="PSUM") as pp, \
                 tc.tile_pool(name="sp", bufs=2, space="PSUM") as sp, \
                 tc.tile_pool(name="tA", bufs=1, space="PSUM") as ptA, \
                 tc.tile_pool(name="tB", bufs=1, space="PSUM") as ptB:

                qh = cp.tile([128, NT, SEQ], bf, name="qh")
                kh = cp.tile([128, NT, SEQ], bf, name="kh")
                vsb = cp.tile([128, NKT, HPC * 65], bf, name="vsb")
                probs = cp.tile([128, 2, NKT, 512], bf, name="probs")
                w_sb = {n: cp.tile([128, NE, DH], bf, name=f"w_{n}") for n in "qkv"}
                b_sb = {n: cp.tile([128, NT], f32, name=f"b_{n}") for n in "qk"}
                cs = cp.tile([128, SEQ], bf, name="cs")
                sn = cp.tile([128, SEQ], bf, name="sn")
                msk = cp.tile([128, 128], bf, name="mskt")
                psw = cp.tile([128, 128], bf, name="psw")

                # DMA order: weights ride with their x h0 tiles (q first so
                # the first matmul starts ~1us in, then k for scores, then
                # v); h1 tiles last -- rounds 0-1 only touch h0, so the x
                # pool recycles h0 bufs into h1 as rounds retire.
                wd = {"q": wq_d, "k": wk_d, "v": wv_d}
                xd = {"q": xq_d, "k": xk_d, "v": xv_d}
                xh = {nm: {} for nm in "qkv"}

                def load_x(nm, sc):
                    t = xp.tile([128, NE, 512], bf, tag="x", name=f"x{nm}_{sc}")
                    nc.sync.dma_start(
                        out=t[:, :, :],
                        in_=xd[nm].rearrange("(e p) s -> p e s", p=128)
                        [:, :, sc * 512:(sc + 1) * 512])
                    xh[nm][sc] = t

                def load_w(nm):
                    nc.sync.dma_start(
                        out=w_sb[nm][:, :, :],
                        in_=wd[nm].rearrange("(e p) d -> p e d", p=128))

                load_w("k")
                load_x("k", 0)
                nc.sync.dma_start(out=b_sb["q"][:, :], in_=bqc_d[:, :])
                nc.sync.dma_start(out=b_sb["k"][:, :], in_=bkc_d[:, :])
                nc.sync.dma_start(out=cs[:, :], in_=cs_d[:, :])
                nc.sync.dma_start(out=sn[:, :], in_=sn_d[:, :])
                nc.sync.dma_start(out=msk[:, :], in_=mk_d[:, 0:128])
                nc.sync.dma_start(out=psw[:, :], in_=pw_d[:, :])
                load_x("k", 1)
                load_x("k", 2)
                load_x("k", 3)
                load_w("q")
                load_x("q", 3)
                load_w("v")
                for sc in range(4):
                    load_x("v", sc)
                # only the ones-columns (col 64 of each 65-block) need init;
                # the v evictions overwrite the 64 data columns of every block
                nc.vector.memset(
                    vsb[:, :, :].rearrange("p k (h u) -> p k h u", u=65)[:, :, :, 64:65],
                    1.0)

                # ---- projections + rope (single full-width matmuls), one
                # 512-col seq chunk at a time ----
                pend = []

                def flush(n=None):
                    k = len(pend) if n is None else max(0, min(n, len(pend)))
                    for _ in range(k):
                        pend.pop(0)()
                def proj_qk_chunk(nm, xt, m, sc, dst):
                    # m-tile (128 out dims) x seq chunk sc (512 cols)
                    flush(len(pend) - 1)  # depth-1 swap deferral
                    cc = slice(sc * 512, (sc + 1) * 512)
                    ps = pp.tile([128, 512], f32, tag="p", name=f"pp{nm}{m}{sc}")
                    for e in range(NE):
                        nc.tensor.matmul(
                            ps[:, :],
                            w_sb[nm][:, e, m * 128:(m + 1) * 128],
                            xt[sc][:, e, :],
                            start=(e == 0), stop=(e == NE - 1))
                    tmp = rp.tile([128, 512], bf, tag="tmp", bufs=4,
                                  name=f"tp{nm}{m}{sc}")
                    nc.vector.tensor_scalar_add(tmp[:, :], ps[:, :],
                                                b_sb[nm][:, m:m + 1])
                    # 32-row block swap (0<->1, 2<->3) via a PE permutation
                    # matmul (psw) -- keeps DMA dispatch off the SP queue; the
                    # swap matmul is deferred one chunk so the PE never waits
                    # head-of-line on this chunk's PSUM eviction
                    nc.vector.tensor_tensor(dst[:, m, cc], tmp[:, :], cs[:, cc], MUL)

                    def do_swap():
                        ps2 = pp.tile([128, 512], f32, tag="p",
                                      name=f"sw{nm}{m}{sc}")
                        nc.tensor.matmul(ps2[:, :], psw[:, :], tmp[:, :],
                                         start=True, stop=True)
                        m2 = rp.tile([128, 512], bf, tag="m2", name=f"m2{nm}{m}{sc}")
                        nc.vector.tensor_tensor(m2[:, :], ps2[:, :], sn[:, cc], MUL)
                        nc.vector.tensor_tensor(dst[:, m, cc], dst[:, m, cc],
                                                m2[:, :], ADD)
                    pend.append(do_swap)

                def proj_v_st(st):
                    sc, o = st // 4, (st % 4) * 128
                    ps = pp.tile([128, 512], f32, tag="p", name=f"ppv{st}")
                    for e in range(NE):
                        nc.tensor.matmul(
                            ps[:, :],
                            xh["v"][sc][:, e, o:o + 128],
                            w_sb["v"][:, e, :],
                            start=(e == 0), stop=(e == NE - 1))
                    nc.vector.tensor_copy(
                        vsb[:, st, :]
                        .rearrange("p (h u) -> p h u", u=65)[:, :, 0:64],
                        ps[:, :].rearrange("p (h d) -> p h d", d=64))

                # row 0 units get dedicated probs slots (4t+kt) so all four
                # can have scores in flight before any v data exists; later
                # rows share slots kt (Tile serializes WAR per slot)
                def scores_kt(t, j, kt, so):
                    # diagonal tile dd: cols < 128*dd are fully masked ->
                    # compute only cols lo..512 in scores/exp/PV; the band
                    # [lo, lo+128) gets the triangular mask; cols < lo keep
                    # stale slot data that pv_kt never reads
                    dd = kt - 4 * j if causal else -1
                    lo = 128 * dd if dd > 0 else 0
                    ps = sp.tile([128, 1024], f32, tag="s", name=f"ps{t}{j}_{kt}")
                    # A/B heads write the tile's two different PSUM banks
                    # from PE row groups 0/1 -> they run concurrently.
                    for half in (0, 1):
                        po = half * 64
                        nc.tensor.matmul(
                            ps[:, half * 512 + lo:(half + 1) * 512],
                            kh[po:po + 64, t, kt * 128:(kt + 1) * 128],
                            qh[po:po + 64, t, j * 512 + lo:(j + 1) * 512],
                            start=True, stop=True)
                    nc.scalar.activation(
                        probs[:, :, so + kt, lo:512],
                        ps[:, :].rearrange("p (h u) -> p h u", h=2)[:, :, lo:512],
                        EXP)
                    if causal and dd >= 0:
                        for half in (0, 1):
                            nc.vector.tensor_tensor(
                                probs[:, half, so + kt, lo:lo + 128],
                                probs[:, half, so + kt, lo:lo + 128],
                                msk[:, :], MUL)

                def pv_kt(t, j, kt, so, pt, nkt):
                    dd = kt - 4 * j if causal else -1
                    lo = 128 * dd if (dd > 0 and kt > 0) else 0
                    for half in (0, 1):
                        lh = 2 * t + half
                        nc.tensor.matmul(
                            pt[half][:, lo:512],
                            vsb[:, kt, lh * 65:(lh + 1) * 65],
                            probs[:, half, so + kt, lo:512],
                            start=(kt == 0), stop=(kt == nkt - 1))

                def unit_out(t, j, pt):
                    for half in (0, 1):
                        lh = 2 * t + half
                        ost = op.tile([65, 512], f32, tag="ost", name=f"os{half}_{t}{j}")
                        nc.vector.tensor_copy(ost[:, :], pt[half][:, :])
                        nc.sync.dma_start(
                            out=outT_d[lh * 64:(lh + 1) * 64, j * 512:(j + 1) * 512],
                            in_=ost[0:64, :])
                        nc.sync.dma_start(
                            out=l_d[lh:lh + 1, j * 512:(j + 1) * 512],
                            in_=ost[64:65, :])

                def pv_unit(t, j, so):
                    nkt = 4 * (j + 1) if causal else NKT
                    pt = {0: ptA.tile([65, 512], f32, tag="t0", name=f"pt0_{t}{j}"),
                          1: ptB.tile([65, 512], f32, tag="t1", name=f"pt1_{t}{j}")}
                    for kt in range(nkt):
                        pv_kt(t, j, kt, so, pt, nkt)
                    unit_out(t, j, pt)

                def unit(t, j):
                    nkt = 4 * (j + 1) if causal else NKT
                    pt = {0: ptA.tile([65, 512], f32, tag="t0", name=f"pt0_{t}{j}"),
                          1: ptB.tile([65, 512], f32, tag="t1", name=f"pt1_{t}{j}")}
                    # scores run one kt ahead of PV so the PE never waits
                    # head-of-line on the exp of the tile it just produced
                    scores_kt(t, j, 0, 0)
                    for kt in range(1, nkt):
                        scores_kt(t, j, kt, 0)
                        pv_kt(t, j, kt - 1, 0, pt, nkt)
                    pv_kt(t, j, nkt - 1, 0, pt, nkt)
                    unit_out(t, j, pt)

                # pipeline by seq chunk: round j projects chunk j of q/k and
                # v seq-tiles 4j..4j+3; unit (t, j) needs exactly chunks <= j,
                # so row-j units interleave with round-(j+1) projections and
                # ScalarE (exp) saturates from ~7us on
                # ---- schedule: all k projections first, then rows of
                # units in DESCENDING j (3..0) -- the exp-heavy rows overlap
                # the v/q projections and the exp-light row 0 drains the
                # tail.  Each unit's PV stream is zipped into the next
                # unit's scores emission (pv of slot kt right before the
                # next scores write of slot kt), so the PE always has
                # exp-independent work and ScalarE stays saturated.
                pvq = []

                def section(t, j, extra=()):
                    n = 4 * (j + 1) if causal else NKT
                    prev = pvq[:]
                    del pvq[:]
                    pt = {0: ptA.tile([65, 512], f32, tag="t0", name=f"pt0_{t}{j}"),
                          1: ptB.tile([65, 512], f32, tag="t1", name=f"pt1_{t}{j}")}
                    ex = list(extra)
                    for kt in range(n):
                        if kt < len(prev):
                            prev[kt]()
                        elif ex:
                            ex.pop(0)()
                        scores_kt(t, j, kt, 0)
                    for c in prev[n:]:
                        c()
                    for c in ex:
                        c()
                    for kt in range(n):
                        pvq.append(lambda kt=kt: pv_kt(t, j, kt, 0, pt, n))
                    pvq.append(lambda: unit_out(t, j, pt))

                for t in range(NT):
                    for sc in range(NQC):
                        proj_qk_chunk("k", xh["k"], t, sc, kh)
                    proj_qk_chunk("q", xh["q"], t, 3, qh)
                    flush()
                    if t == 0:
                        # v projections ride inside the first section's
                        # exp-paced scores stream
                        section(0, 3, [lambda st=st: proj_v_st(st)
                                       for st in range(NKT)])
                    else:
                        section(t, 3)
                    if t == 1:
                        load_x("q", 2)
                    if t == 3:
                        load_x("q", 1)
                for j in (2, 1, 0):
                    for t in range(NT):
                        proj_qk_chunk("q", xh["q"], t, j, qh)
                        flush()
                        section(t, j)
                        if j == 2 and t == 1:
                            load_x("q", 0)
                for c in pvq:
                    c()
    _built[causal] = nc
    nc.compile()
    return nc


def _prep_core_inputs(c, q, k, v, Wq, bq, Wk, bk, Wv, bv, sin, cos):
    b, hh = c // 2, c % 2
    hs = slice(hh * DH, (hh + 1) * DH)

    perm = np.empty(DH, np.int64)
    for lh in range(HPC):
        base = (hh * HPC + lh) * HD
        perm[lh * HD:lh * HD + HALF] = base + 2 * np.arange(HALF)
        perm[lh * HD + HALF:(lh + 1) * HD] = base + 2 * np.arange(HALF) + 1

    s = 0.125  # 1/sqrt(HD), folded into the q projection
    wq = np.ascontiguousarray((Wq[perm, :] * s).T).astype(BF16)
    wk = np.ascontiguousarray(Wk[perm, :].T).astype(BF16)
    wv = np.ascontiguousarray(Wv[hs, :].T).astype(BF16)

    p32 = np.arange(128) % 32
    cs2 = cos[:, p32].T.astype(BF16)
    sgn = np.where((np.arange(128) // 32) % 2 == 0, -1.0, 1.0).astype(np.float32)
    sn2 = (sin[:, p32] * sgn[None, :]).T.astype(BF16)

    kk = np.arange(128)[:, None]
    msk = (kk <= kk.T).astype(BF16)  # [128,128] lower-tri in q-major sense
    psw = np.zeros((128, 128), np.float32)
    psw[np.arange(128), np.arange(128) ^ 32] = 1.0

    return {
        "xq": np.ascontiguousarray(q[b].T).astype(BF16),
        "xk": np.ascontiguousarray(k[b].T).astype(BF16),
        "xv": np.ascontiguousarray(v[b].T).astype(BF16),
        "wq": wq, "wk": wk, "wv": wv,
        "bqc": np.ascontiguousarray((bq[perm] * s).reshape(NT, 128).T, np.float32),
        "bkc": np.ascontiguousarray(bk[perm].reshape(NT, 128).T, np.float32),
        "cs2": cs2, "sn2": sn2, "msk": msk, "psw": psw.astype(BF16),
    }


def prep_in_maps(q, k, v, Wq, bq, Wk, bk, Wv, bv, sin, cos):
    args = [np.asarray(a, np.float32) for a in (q, k, v, Wq, bq, Wk, bk, Wv, bv, sin, cos)]
    maps = [_prep_core_inputs(c, *args) for c in range(8)]
    return maps, args[8]  # bv needed on host in assemble()


def assemble(results, bv):
    out = np.empty((BATCH, SEQ, EMB), np.float32)
    for c in range(8):
        b, hh = c // 2, c % 2
        outT = np.asarray(results[c]["outT"], np.float32)
        l = np.asarray(results[c]["lsum"], np.float32)
        a = outT.reshape(HPC, HD, SEQ) / l[:, None, :]
        out[b, :, hh * DH:(hh + 1) * DH] = a.reshape(DH, SEQ).T \
            + bv[hh * DH:(hh + 1) * DH][None, :]
    return out


def run(in_maps, causal=True, trace=False, **kw):
    _install_ntff_shim()
    from concourse.bass_utils import run_bass_kernel_spmd
    nc = build(causal)
    return run_bass_kernel_spmd(nc, in_maps, core_ids=list(range(8)), trace=trace, **kw)


def kernel(q, k, v, Wq, bq, Wk, bk, Wv, bv, sin, cos, mask):
    in_maps, bv_f = prep_in_maps(q, k, v, Wq, bq, Wk, bk, Wv, bv, sin, cos)
    r = run(in_maps, causal=bool(mask))
    return assemble(r.results, bv_f)



# revision 26
# speedup vs baseline: 1.1482x; 1.1482x over previous
"""Multi-head causal attention (QKV proj + RoPE + softmax) on 8 TRN2 NeuronCores.

Sharding: batch 4-way x head-group 2-way -> each core handles 1 batch and 8
contiguous heads (512 output channels). No collectives; host gathers slices.

Per-core algorithm (all matmul compute in bf16, fp32 PSUM accumulation):
  - host passes x.T (q/k/v of its batch, transposed to [emb, seq]) and W.T
    shards so every matmul contracts over the partition dim without on-device
    transposes.
  - q/k weights are row-permuted per head into [even dims | odd dims] so RoPE
    becomes: rot = x*cs + swap32(x)*sn, where swap32 is an SBUF partition-block
    swap done by DMA. The per-head dim permutation cancels in q.k dot products.
  - q/k biases are per-partition columns folded into the PSUM eviction
    (tensor_scalar add); the v bias is applied on host: P@(V+b) = P@V + l*b.
  - scores are computed transposed, S_T[k, q] = kh_T.T @ qh_T (K=64
    contraction; the A/B heads of a 128-row tile are emitted adjacently so
    they run concurrently on PE row groups).
  - softmax: exp on ScalarE from PSUM at [128,1024] granularity (no max
    subtraction: |scores| <= ~5 by construction), causal mask multiplies on
    GpSimd for the diagonal pairs only; fully-masked k-tiles are skipped.
  - attnT[d, q] = sum_kt V_tile[k,d|1].T @ P_T[k, q] -- a ones-column appended
    to V makes row 64 the softmax denominator for free.
  - unnormalized attnT and the denominator row go to HBM via one SBUF staging
    copy; division + final transpose + v-bias happen on host.
"""

import sys
import types

import numpy as np
import ml_dtypes

BF16 = ml_dtypes.bfloat16
SEQ, EMB, NHEADS, BATCH = 2048, 1024, 16, 4
HD, HALF = 64, 32
HPC = 8          # heads per core
DH = 512         # output dims per core
NE = EMB // 128  # 8 contraction tiles
NT = 4           # head-pair (128-row) dout tiles
NKT = SEQ // 128  # 16 key tiles
NQC = SEQ // 512  # 4 query chunks


def _install_ntff_shim():
    """The image's antenv lacks axon_hooks; synthesize it from trn_agent_boot
    so run_bass_kernel_spmd(trace=True) can profile. Harmless if unused."""
    try:
        import antenv.axon_hooks  # noqa: F401
        return
    except ImportError:
        pass
    try:
        from trn_agent_boot.trn_boot import _ntff_profile_via_ctypes
        import antenv
    except ImportError:
        return
    hook = _ntff_profile_via_ctypes("/opt/axon/libaxon_pjrt.so")
    mod = types.ModuleType("antenv.axon_hooks")
    mod.get_axon_ntff_profile_hook = lambda: hook
    mod.set_axon_ntff_profile_hook = lambda h: None
    sys.modules["antenv.axon_hooks"] = mod
    antenv.axon_hooks = mod


_built = {}


def build(causal=True):
    if causal in _built:
        return _built[causal]
    import concourse.mybir as mybir
    import concourse.tile as tile
    from concourse import bacc

    f32 = mybir.dt.float32
    bf = mybir.dt.bfloat16
    EXP = mybir.ActivationFunctionType.Exp
    MUL = mybir.AluOpType.mult
    ADD = mybir.AluOpType.add

    nc = bacc.Bacc(None, target_bir_lowering=False, debug=False)
    with tile.TileContext(nc) as tc:
        with tc.tile_pool(name="dram", bufs=1, space="DRAM") as dram:
            xq_d = dram.tile([EMB, SEQ], bf, kind="ExternalInput", name="xq", uniquify=False)
            xk_d = dram.tile([EMB, SEQ], bf, kind="ExternalInput", name="xk", uniquify=False)
            xv_d = dram.tile([EMB, SEQ], bf, kind="ExternalInput", name="xv", uniquify=False)
            wq_d = dram.tile([EMB, DH], bf, kind="ExternalInput", name="wq", uniquify=False)
            wk_d = dram.tile([EMB, DH], bf, kind="ExternalInput", name="wk", uniquify=False)
            wv_d = dram.tile([EMB, DH], bf, kind="ExternalInput", name="wv", uniquify=False)
            bqc_d = dram.tile([128, NT], f32, kind="ExternalInput", name="bqc", uniquify=False)
            bkc_d = dram.tile([128, NT], f32, kind="ExternalInput", name="bkc", uniquify=False)
            cs_d = dram.tile([128, SEQ], bf, kind="ExternalInput", name="cs2", uniquify=False)
            sn_d = dram.tile([128, SEQ], bf, kind="ExternalInput", name="sn2", uniquify=False)
            mk_d = dram.tile([128, 128], bf, kind="ExternalInput", name="msk", uniquify=False)
            pw_d = dram.tile([128, 128], bf, kind="ExternalInput", name="psw", uniquify=False)
            outT_d = dram.tile([DH, SEQ], f32, kind="ExternalOutput", name="outT", uniquify=False)
            l_d = dram.tile([HPC, SEQ], f32, kind="ExternalOutput", name="lsum", uniquify=False)

            with tc.tile_pool(name="const", bufs=1) as cp, \
                 tc.tile_pool(name="xin", bufs=10) as xp, \
                 tc.tile_pool(name="rope", bufs=2) as rp, \
                 tc.tile_pool(name="ostage", bufs=2) as op, \
                 tc.tile_pool(name="pp", bufs=2, space="PSUM") as pp, \
                 tc.tile_pool(name="sp", bufs=2, space="PSUM") as sp, \
                 tc.tile_pool(name="tA", bufs=1, space="PSUM") as ptA, \
                 tc.tile_pool(name="tB", bufs=1, space="PSUM") as ptB:

                qh = cp.tile([128, NT, SEQ], bf, name="qh")
                kh = cp.tile([128, NT, SEQ], bf, name="kh")
                vsb = cp.tile([128, NKT, HPC * 65], bf, name="vsb")
                probs = cp.tile([128, 2, NKT, 512], bf, name="probs")
                w_sb = {n: cp.tile([128, NE, DH], bf, name=f"w_{n}") for n in "qkv"}
                b_sb = {n: cp.tile([128, NT], f32, name=f"b_{n}") for n in "qk"}
                cs = cp.tile([128, SEQ], bf, name="cs")
                sn = cp.tile([128, SEQ], bf, name="sn")
                msk = cp.tile([128, 128], bf, name="mskt")
                psw = cp.tile([128, 128], bf, name="psw")

                # DMA order: weights ride with their x h0 tiles (q first so
                # the first matmul starts ~1us in, then k for scores, then
                # v); h1 tiles last -- rounds 0-1 only touch h0, so the x
                # pool recycles h0 bufs into h1 as rounds retire.
                wd = {"q": wq_d, "k": wk_d, "v": wv_d}
                xd = {"q": xq_d, "k": xk_d, "v": xv_d}
                xh = {nm: {} for nm in "qkv"}

                def load_x(nm, sc):
                    t = xp.tile([128, NE, 512], bf, tag="x", name=f"x{nm}_{sc}")
                    nc.sync.dma_start(
                        out=t[:, :, :],
                        in_=xd[nm].rearrange("(e p) s -> p e s", p=128)
                        [:, :, sc * 512:(sc + 1) * 512])
                    xh[nm][sc] = t

                def load_w(nm):
                    nc.sync.dma_start(
                        out=w_sb[nm][:, :, :],
                        in_=wd[nm].rearrange("(e p) d -> p e d", p=128))

                load_w("k")
                load_x("k", 0)
                nc.sync.dma_start(out=b_sb["q"][:, :], in_=bqc_d[:, :])
                nc.sync.dma_start(out=b_sb["k"][:, :], in_=bkc_d[:, :])
                nc.sync.dma_start(out=cs[:, :], in_=cs_d[:, :])
                nc.sync.dma_start(out=sn[:, :], in_=sn_d[:, :])
                nc.sync.dma_start(out=msk[:, :], in_=mk_d[:, 0:128])
                nc.sync.dma_start(out=psw[:, :], in_=pw_d[:, :])
                load_x("k", 1)
                load_x("k", 2)
                load_x("k", 3)
                load_w("q")
                load_x("q", 3)
                load_w("v")
                for sc in range(4):
                    load_x("v", sc)
                # only the ones-columns (col 64 of each 65-block) need init;
                # the v evictions overwrite the 64 data columns of every block
                nc.vector.memset(
                    vsb[:, :, :].rearrange("p k (h u) -> p k h u", u=65)[:, :, :, 64:65],
                    1.0)
                # warm the PE p-state while the first input DMAs stream: junk
                # matmuls on a memset scratch tile (result never read) keep
                # the PE continuously busy so real work starts at full clock
                jt = cp.tile([128, 512], bf, name="junk")
                nc.vector.memset(jt[:, :], 0.0)
                jp = pp.tile([128, 512], f32, tag="p", name="jpsum")
                for i in range(20):
                    nc.tensor.matmul(jp[:, :], jt[:, 0:128], jt[:, :],
                                     start=(i == 0), stop=(i == 19))

                # ---- projections + rope (single full-width matmuls), one
                # 512-col seq chunk at a time ----
                pend = []

                def flush(n=None):
                    k = len(pend) if n is None else max(0, min(n, len(pend)))
                    for _ in range(k):
                        pend.pop(0)()
                def proj_qk_chunk(nm, xt, m, sc, dst):
                    # m-tile (128 out dims) x seq chunk sc (512 cols)
                    flush(len(pend) - 1)  # depth-1 swap deferral
                    cc = slice(sc * 512, (sc + 1) * 512)
                    ps = pp.tile([128, 512], f32, tag="p", name=f"pp{nm}{m}{sc}")
                    for e in range(NE):
                        nc.tensor.matmul(
                            ps[:, :],
                            w_sb[nm][:, e, m * 128:(m + 1) * 128],
                            xt[sc][:, e, :],
                            start=(e == 0), stop=(e == NE - 1))
                    tmp = rp.tile([128, 512], bf, tag="tmp", bufs=4,
                                  name=f"tp{nm}{m}{sc}")
                    nc.vector.tensor_scalar_add(tmp[:, :], ps[:, :],
                                                b_sb[nm][:, m:m + 1])
                    # 32-row block swap (0<->1, 2<->3) via a PE permutation
                    # matmul (psw) -- keeps DMA dispatch off the SP queue; the
                    # swap matmul is deferred one chunk so the PE never waits
                    # head-of-line on this chunk's PSUM eviction
                    nc.vector.tensor_tensor(dst[:, m, cc], tmp[:, :], cs[:, cc], MUL)

                    def do_swap():
                        ps2 = pp.tile([128, 512], f32, tag="p",
                                      name=f"sw{nm}{m}{sc}")
                        nc.tensor.matmul(ps2[:, :], psw[:, :], tmp[:, :],
                                         start=True, stop=True)
                        m2 = rp.tile([128, 512], bf, tag="m2", name=f"m2{nm}{m}{sc}")
                        nc.vector.tensor_tensor(m2[:, :], ps2[:, :], sn[:, cc], MUL)
                        nc.vector.tensor_tensor(dst[:, m, cc], dst[:, m, cc],
                                                m2[:, :], ADD)
                    pend.append(do_swap)

                def proj_v_st(st):
                    sc, o = st // 4, (st % 4) * 128
                    ps = pp.tile([128, 512], f32, tag="p", name=f"ppv{st}")
                    for e in range(NE):
                        nc.tensor.matmul(
                            ps[:, :],
                            xh["v"][sc][:, e, o:o + 128],
                            w_sb["v"][:, e, :],
                            start=(e == 0), stop=(e == NE - 1))
                    nc.vector.tensor_copy(
                        vsb[:, st, :]
                        .rearrange("p (h u) -> p h u", u=65)[:, :, 0:64],
                        ps[:, :].rearrange("p (h d) -> p h d", d=64))

                # row 0 units get dedicated probs slots (4t+kt) so all four
                # can have scores in flight before any v data exists; later
                # rows share slots kt (Tile serializes WAR per slot)
                def scores_kt(t, j, kt, so):
                    # diagonal tile dd: cols < 128*dd are fully masked ->
                    # compute only cols lo..512 in scores/exp/PV; the band
                    # [lo, lo+128) gets the triangular mask; cols < lo keep
                    # stale slot data that pv_kt never reads
                    dd = kt - 4 * j if causal else -1
                    lo = 128 * dd if dd > 0 else 0
                    ps = sp.tile([128, 1024], f32, tag="s", name=f"ps{t}{j}_{kt}")
                    # A/B heads write the tile's two different PSUM banks
                    # from PE row groups 0/1 -> they run concurrently.
                    for half in (0, 1):
                        po = half * 64
                        nc.tensor.matmul(
                            ps[:, half * 512 + lo:(half + 1) * 512],
                            kh[po:po + 64, t, kt * 128:(kt + 1) * 128],
                            qh[po:po + 64, t, j * 512 + lo:(j + 1) * 512],
                            start=True, stop=True)
                    nc.scalar.activation(
                        probs[:, :, so + kt, lo:512],
                        ps[:, :].rearrange("p (h u) -> p h u", h=2)[:, :, lo:512],
                        EXP)
                    if causal and dd >= 0:
                        for half in (0, 1):
                            nc.vector.tensor_tensor(
                                probs[:, half, so + kt, lo:lo + 128],
                                probs[:, half, so + kt, lo:lo + 128],
                                msk[:, :], MUL)

                def pv_kt(t, j, kt, so, pt, nkt):
                    dd = kt - 4 * j if causal else -1
                    lo = 128 * dd if (dd > 0 and kt > 0) else 0
                    for half in (0, 1):
                        lh = 2 * t + half
                        nc.tensor.matmul(
                            pt[half][:, lo:512],
                            vsb[:, kt, lh * 65:(lh + 1) * 65],
                            probs[:, half, so + kt, lo:512],
                            start=(kt == 0), stop=(kt == nkt - 1))

                def unit_out(t, j, pt):
                    for half in (0, 1):
                        lh = 2 * t + half
                        ost = op.tile([65, 512], f32, tag="ost", name=f"os{half}_{t}{j}")
                        nc.vector.tensor_copy(ost[:, :], pt[half][:, :])
                        nc.sync.dma_start(
                            out=outT_d[lh * 64:(lh + 1) * 64, j * 512:(j + 1) * 512],
                            in_=ost[0:64, :])
                        nc.sync.dma_start(
                            out=l_d[lh:lh + 1, j * 512:(j + 1) * 512],
                            in_=ost[64:65, :])

                def pv_unit(t, j, so):
                    nkt = 4 * (j + 1) if causal else NKT
                    pt = {0: ptA.tile([65, 512], f32, tag="t0", name=f"pt0_{t}{j}"),
                          1: ptB.tile([65, 512], f32, tag="t1", name=f"pt1_{t}{j}")}
                    for kt in range(nkt):
                        pv_kt(t, j, kt, so, pt, nkt)
                    unit_out(t, j, pt)

                def unit(t, j):
                    nkt = 4 * (j + 1) if causal else NKT
                    pt = {0: ptA.tile([65, 512], f32, tag="t0", name=f"pt0_{t}{j}"),
                          1: ptB.tile([65, 512], f32, tag="t1", name=f"pt1_{t}{j}")}
                    # scores run one kt ahead of PV so the PE never waits
                    # head-of-line on the exp of the tile it just produced
                    scores_kt(t, j, 0, 0)
                    for kt in range(1, nkt):
                        scores_kt(t, j, kt, 0)
                        pv_kt(t, j, kt - 1, 0, pt, nkt)
                    pv_kt(t, j, nkt - 1, 0, pt, nkt)
                    unit_out(t, j, pt)

                # pipeline by seq chunk: round j projects chunk j of q/k and
                # v seq-tiles 4j..4j+3; unit (t, j) needs exactly chunks <= j,
                # so row-j units interleave with round-(j+1) projections and
                # ScalarE (exp) saturates from ~7us on
                # ---- schedule: all k projections first, then rows of
                # units in DESCENDING j (3..0) -- the exp-heavy rows overlap
                # the v/q projections and the exp-light row 0 drains the
                # tail.  Each unit's PV stream is zipped into the next
                # unit's scores emission (pv of slot kt right before the
                # next scores write of slot kt), so the PE always has
                # exp-independent work and ScalarE stays saturated.
                pvq = []

                def section(t, j, extra=()):
                    n = 4 * (j + 1) if causal else NKT
                    prev = pvq[:]
                    del pvq[:]
                    pt = {0: ptA.tile([65, 512], f32, tag="t0", name=f"pt0_{t}{j}"),
                          1: ptB.tile([65, 512], f32, tag="t1", name=f"pt1_{t}{j}")}
                    ex = list(extra)
                    for kt in range(n):
                        if kt < len(prev):
                            prev[kt]()
                        if ex and (kt * len(extra)) // n != ((kt + 1) * len(extra)) // n:
                            ex.pop(0)()
                        scores_kt(t, j, kt, 0)
                    for c in prev[n:]:
                        c()
                    for c in ex:
                        c()
                    for kt in range(n):
                        pvq.append(lambda kt=kt: pv_kt(t, j, kt, 0, pt, n))
                    pvq.append(lambda: unit_out(t, j, pt))

                # all v chunks must precede the first pv drain (section
                # (1,3)); later sections take the NEXT row's q chunks as
                # PE filler so every section stays PE-bound (full clock)
                def qc(t, j):
                    return lambda: proj_qk_chunk("q", xh["q"], t, j, qh)

                qfill = {(1, 3): [qc(0, 2)], (2, 3): [qc(1, 2)],
                         (3, 3): [qc(2, 2), qc(3, 2)],
                         (0, 2): [qc(0, 1)], (1, 2): [qc(1, 1)],
                         (2, 2): [qc(2, 1)], (3, 2): [qc(3, 1)],
                         (0, 1): [qc(0, 0)], (1, 1): [qc(1, 0)],
                         (2, 1): [qc(2, 0)], (3, 1): [qc(3, 0)]}
                for t in range(NT):
                    for sc in range(NQC):
                        proj_qk_chunk("k", xh["k"], t, sc, kh)
                    proj_qk_chunk("q", xh["q"], t, 3, qh)
                    flush()
                    ex = ([lambda st=st: proj_v_st(st) for st in range(NKT)]
                          if t == 0 else qfill[(t, 3)])
                    section(t, 3, ex)
                    if t == 0:
                        load_x("q", 2)
                    if t == 2:
                        load_x("q", 1)
                for j in (2, 1, 0):
                    for t in range(NT):
                        if j == 0:
                            flush()
                        section(t, j, qfill.get((t, j), ()))
                        if j == 2 and t == 1:
                            load_x("q", 0)
                for c in pvq:
                    c()
    _built[causal] = nc
    nc.compile()
    return nc


def _prep_core_inputs(c, q, k, v, Wq, bq, Wk, bk, Wv, bv, sin, cos):
    b, hh = c // 2, c % 2
    hs = slice(hh * DH, (hh + 1) * DH)

    perm = np.empty(DH, np.int64)
    for lh in range(HPC):
        base = (hh * HPC + lh) * HD
        perm[lh * HD:lh * HD + HALF] = base + 2 * np.arange(HALF)
        perm[lh * HD + HALF:(lh + 1) * HD] = base + 2 * np.arange(HALF) + 1

    s = 0.125  # 1/sqrt(HD), folded into the q projection
    wq = np.ascontiguousarray((Wq[perm, :] * s).T).astype(BF16)
    wk = np.ascontiguousarray(Wk[perm, :].T).astype(BF16)
    wv = np.ascontiguousarray(Wv[hs, :].T).astype(BF16)

    p32 = np.arange(128) % 32
    cs2 = cos[:, p32].T.astype(BF16)
    sgn = np.where((np.arange(128) // 32) % 2 == 0, -1.0, 1.0).astype(np.float32)
    sn2 = (sin[:, p32] * sgn[None, :]).T.astype(BF16)

    kk = np.arange(128)[:, None]
    msk = (kk <= kk.T).astype(BF16)  # [128,128] lower-tri in q-major sense
    psw = np.zeros((128, 128), np.float32)
    psw[np.arange(128), np.arange(128) ^ 32] = 1.0

    return {
        "xq": np.ascontiguousarray(q[b].T).astype(BF16),
        "xk": np.ascontiguousarray(k[b].T).astype(BF16),
        "xv": np.ascontiguousarray(v[b].T).astype(BF16),
        "wq": wq, "wk": wk, "wv": wv,
        "bqc": np.ascontiguousarray((bq[perm] * s).reshape(NT, 128).T, np.float32),
        "bkc": np.ascontiguousarray(bk[perm].reshape(NT, 128).T, np.float32),
        "cs2": cs2, "sn2": sn2, "msk": msk, "psw": psw.astype(BF16),
    }


def prep_in_maps(q, k, v, Wq, bq, Wk, bk, Wv, bv, sin, cos):
    args = [np.asarray(a, np.float32) for a in (q, k, v, Wq, bq, Wk, bk, Wv, bv, sin, cos)]
    maps = [_prep_core_inputs(c, *args) for c in range(8)]
    return maps, args[8]  # bv needed on host in assemble()


def assemble(results, bv):
    out = np.empty((BATCH, SEQ, EMB), np.float32)
    for c in range(8):
        b, hh = c // 2, c % 2
        outT = np.asarray(results[c]["outT"], np.float32)
        l = np.asarray(results[c]["lsum"], np.float32)
        a = outT.reshape(HPC, HD, SEQ) / l[:, None, :]
        out[b, :, hh * DH:(hh + 1) * DH] = a.reshape(DH, SEQ).T \
            + bv[hh * DH:(hh + 1) * DH][None, :]
    return out


def run(in_maps, causal=True, trace=False, **kw):
    _install_ntff_shim()
    from concourse.bass_utils import run_bass_kernel_spmd
    nc = build(causal)
    return run_bass_kernel_spmd(nc, in_maps, core_ids=list(range(8)), trace=trace, **kw)


def kernel(q, k, v, Wq, bq, Wk, bk, Wv, bv, sin, cos, mask):
    in_maps, bv_f = prep_in_maps(q, k, v, Wq, bq, Wk, bk, Wv, bv, sin, cos)
    r = run(in_maps, causal=bool(mask))
    return assemble(r.results, bv_f)



# revision 27
# speedup vs baseline: 1.1706x; 1.0194x over previous
"""Multi-head causal attention (QKV proj + RoPE + softmax) on 8 TRN2 NeuronCores.

Sharding: batch 4-way x head-group 2-way -> each core handles 1 batch and 8
contiguous heads (512 output channels). No collectives; host gathers slices.

Per-core algorithm (all matmul compute in bf16, fp32 PSUM accumulation):
  - host passes x.T (q/k/v of its batch, transposed to [emb, seq]) and W.T
    shards so every matmul contracts over the partition dim without on-device
    transposes.
  - q/k weights are row-permuted per head into [even dims | odd dims] so RoPE
    becomes: rot = x*cs + swap32(x)*sn, where swap32 is an SBUF partition-block
    swap done by DMA. The per-head dim permutation cancels in q.k dot products.
  - q/k biases are per-partition columns folded into the PSUM eviction
    (tensor_scalar add); the v bias is applied on host: P@(V+b) = P@V + l*b.
  - scores are computed transposed, S_T[k, q] = kh_T.T @ qh_T (K=64
    contraction; the A/B heads of a 128-row tile are emitted adjacently so
    they run concurrently on PE row groups).
  - softmax: exp on ScalarE from PSUM at [128,1024] granularity (no max
    subtraction: |scores| <= ~5 by construction), causal mask multiplies on
    GpSimd for the diagonal pairs only; fully-masked k-tiles are skipped.
  - attnT[d, q] = sum_kt V_tile[k,d|1].T @ P_T[k, q] -- a ones-column appended
    to V makes row 64 the softmax denominator for free.
  - unnormalized attnT and the denominator row go to HBM via one SBUF staging
    copy; division + final transpose + v-bias happen on host.
"""

import sys
import types

import numpy as np
import ml_dtypes

BF16 = ml_dtypes.bfloat16
SEQ, EMB, NHEADS, BATCH = 2048, 1024, 16, 4
HD, HALF = 64, 32
HPC = 8          # heads per core
DH = 512         # output dims per core
NE = EMB // 128  # 8 contraction tiles
NT = 4           # head-pair (128-row) dout tiles
NKT = SEQ // 128  # 16 key tiles
NQC = SEQ // 512  # 4 query chunks


def _install_ntff_shim():
    """The image's antenv lacks axon_hooks; synthesize it from trn_agent_boot
    so run_bass_kernel_spmd(trace=True) can profile. Harmless if unused."""
    try:
        import antenv.axon_hooks  # noqa: F401
        return
    except ImportError:
        pass
    try:
        from trn_agent_boot.trn_boot import _ntff_profile_via_ctypes
        import antenv
    except ImportError:
        return
    hook = _ntff_profile_via_ctypes("/opt/axon/libaxon_pjrt.so")
    mod = types.ModuleType("antenv.axon_hooks")
    mod.get_axon_ntff_profile_hook = lambda: hook
    mod.set_axon_ntff_profile_hook = lambda h: None
    sys.modules["antenv.axon_hooks"] = mod
    antenv.axon_hooks = mod


_built = {}


def build(causal=True):
    if causal in _built:
        return _built[causal]
    import concourse.mybir as mybir
    import concourse.tile as tile
    from concourse import bacc

    f32 = mybir.dt.float32
    bf = mybir.dt.bfloat16
    EXP = mybir.ActivationFunctionType.Exp
    MUL = mybir.AluOpType.mult
    ADD = mybir.AluOpType.add

    nc = bacc.Bacc(None, target_bir_lowering=False, debug=False)
    with tile.TileContext(nc) as tc:
        with tc.tile_pool(name="dram", bufs=1, space="DRAM") as dram:
            xq_d = dram.tile([EMB, SEQ], bf, kind="ExternalInput", name="xq", uniquify=False)
            xk_d = dram.tile([EMB, SEQ], bf, kind="ExternalInput", name="xk", uniquify=False)
            xv_d = dram.tile([EMB, SEQ], bf, kind="ExternalInput", name="xv", uniquify=False)
            wq_d = dram.tile([EMB, DH], bf, kind="ExternalInput", name="wq", uniquify=False)
            wk_d = dram.tile([EMB, DH], bf, kind="ExternalInput", name="wk", uniquify=False)
            wv_d = dram.tile([EMB, DH], bf, kind="ExternalInput", name="wv", uniquify=False)
            bqc_d = dram.tile([128, NT], f32, kind="ExternalInput", name="bqc", uniquify=False)
            bkc_d = dram.tile([128, NT], f32, kind="ExternalInput", name="bkc", uniquify=False)
            cs_d = dram.tile([128, SEQ], bf, kind="ExternalInput", name="cs2", uniquify=False)
            sn_d = dram.tile([128, SEQ], bf, kind="ExternalInput", name="sn2", uniquify=False)
            mk_d = dram.tile([128, 128], bf, kind="ExternalInput", name="msk", uniquify=False)
            pw_d = dram.tile([128, 128], bf, kind="ExternalInput", name="psw", uniquify=False)
            outT_d = dram.tile([DH, SEQ], f32, kind="ExternalOutput", name="outT", uniquify=False)
            l_d = dram.tile([HPC, SEQ], f32, kind="ExternalOutput", name="lsum", uniquify=False)

            with tc.tile_pool(name="const", bufs=1) as cp, \
                 tc.tile_pool(name="xin", bufs=10) as xp, \
                 tc.tile_pool(name="rope", bufs=2) as rp, \
                 tc.tile_pool(name="ostage", bufs=2) as op, \
                 tc.tile_pool(name="pp", bufs=2, space="PSUM") as pp, \
                 tc.tile_pool(name="sp", bufs=2, space="PSUM") as sp, \
                 tc.tile_pool(name="tA", bufs=1, space="PSUM") as ptA, \
                 tc.tile_pool(name="tB", bufs=1, space="PSUM") as ptB:

                qh = cp.tile([128, NT, SEQ], bf, name="qh")
                kh = cp.tile([128, NT, SEQ], bf, name="kh")
                vsb = cp.tile([128, NKT, HPC * 65], bf, name="vsb")
                probs = cp.tile([128, 2, NKT, 512], bf, name="probs")
                w_sb = {n: cp.tile([128, NE, DH], bf, name=f"w_{n}") for n in "qkv"}
                b_sb = {n: cp.tile([128, NT], f32, name=f"b_{n}") for n in "qk"}
                cs = cp.tile([128, SEQ], bf, name="cs")
                sn = cp.tile([128, SEQ], bf, name="sn")
                msk = cp.tile([128, 128], bf, name="mskt")
                psw = cp.tile([128, 128], bf, name="psw")

                # DMA order: weights ride with their x h0 tiles (q first so
                # the first matmul starts ~1us in, then k for scores, then
                # v); h1 tiles last -- rounds 0-1 only touch h0, so the x
                # pool recycles h0 bufs into h1 as rounds retire.
                wd = {"q": wq_d, "k": wk_d, "v": wv_d}
                xd = {"q": xq_d, "k": xk_d, "v": xv_d}
                xh = {nm: {} for nm in "qkv"}

                def load_x(nm, sc):
                    t = xp.tile([128, NE, 512], bf, tag="x", name=f"x{nm}_{sc}")
                    nc.sync.dma_start(
                        out=t[:, :, :],
                        in_=xd[nm].rearrange("(e p) s -> p e s", p=128)
                        [:, :, sc * 512:(sc + 1) * 512])
                    xh[nm][sc] = t

                def load_w(nm):
                    nc.sync.dma_start(
                        out=w_sb[nm][:, :, :],
                        in_=wd[nm].rearrange("(e p) d -> p e d", p=128))

                load_w("k")
                load_x("k", 0)
                nc.sync.dma_start(out=b_sb["q"][:, :], in_=bqc_d[:, :])
                nc.sync.dma_start(out=b_sb["k"][:, :], in_=bkc_d[:, :])
                nc.sync.dma_start(out=cs[:, :], in_=cs_d[:, :])
                nc.sync.dma_start(out=sn[:, :], in_=sn_d[:, :])
                nc.sync.dma_start(out=msk[:, :], in_=mk_d[:, 0:128])
                nc.sync.dma_start(out=psw[:, :], in_=pw_d[:, :])
                load_x("k", 1)
                load_x("k", 2)
                load_x("k", 3)
                load_w("q")
                load_x("q", 3)
                load_w("v")
                for sc in range(4):
                    load_x("v", sc)
                # only the ones-columns (col 64 of each 65-block) need init;
                # the v evictions overwrite the 64 data columns of every block
                nc.vector.memset(
                    vsb[:, :, :].rearrange("p k (h u) -> p k h u", u=65)[:, :, :, 64:65],
                    1.0)
                # warm the PE p-state while the first input DMAs stream: junk
                # matmuls on a memset scratch tile (result never read) keep
                # the PE continuously busy so real work starts at full clock
                jt = cp.tile([128, 512], bf, name="junk")
                nc.vector.memset(jt[:, :], 0.0)
                jp = pp.tile([128, 512], f32, tag="p", name="jpsum")
                for i in range(20):
                    nc.tensor.matmul(jp[:, :], jt[:, 0:128], jt[:, :],
                                     start=(i == 0), stop=(i == 19))

                # ---- projections + rope (single full-width matmuls), one
                # 512-col seq chunk at a time ----
                pend = []

                def flush(n=None):
                    k = len(pend) if n is None else max(0, min(n, len(pend)))
                    for _ in range(k):
                        pend.pop(0)()
                def proj_qk_chunk(nm, xt, m, sc, dst):
                    # m-tile (128 out dims) x seq chunk sc (512 cols)
                    flush(len(pend) - 1)  # depth-1 swap deferral
                    cc = slice(sc * 512, (sc + 1) * 512)
                    ps = pp.tile([128, 512], f32, tag="p", name=f"pp{nm}{m}{sc}")
                    for e in range(NE):
                        nc.tensor.matmul(
                            ps[:, :],
                            w_sb[nm][:, e, m * 128:(m + 1) * 128],
                            xt[sc][:, e, :],
                            start=(e == 0), stop=(e == NE - 1))
                    tmp = rp.tile([128, 512], bf, tag="tmp", bufs=4,
                                  name=f"tp{nm}{m}{sc}")
                    nc.vector.tensor_scalar_add(tmp[:, :], ps[:, :],
                                                b_sb[nm][:, m:m + 1])
                    # 32-row block swap (0<->1, 2<->3) via a PE permutation
                    # matmul (psw) -- keeps DMA dispatch off the SP queue; the
                    # swap matmul is deferred one chunk so the PE never waits
                    # head-of-line on this chunk's PSUM eviction
                    nc.vector.tensor_tensor(dst[:, m, cc], tmp[:, :], cs[:, cc], MUL)

                    def do_swap():
                        ps2 = pp.tile([128, 512], f32, tag="p",
                                      name=f"sw{nm}{m}{sc}")
                        nc.tensor.matmul(ps2[:, :], psw[:, :], tmp[:, :],
                                         start=True, stop=True)
                        m2 = rp.tile([128, 512], bf, tag="m2", name=f"m2{nm}{m}{sc}")
                        nc.vector.tensor_tensor(m2[:, :], ps2[:, :], sn[:, cc], MUL)
                        nc.vector.tensor_tensor(dst[:, m, cc], dst[:, m, cc],
                                                m2[:, :], ADD)
                    pend.append(do_swap)

                def proj_v_st(st):
                    sc, o = st // 4, (st % 4) * 128
                    ps = pp.tile([128, 512], f32, tag="p", name=f"ppv{st}")
                    for e in range(NE):
                        nc.tensor.matmul(
                            ps[:, :],
                            xh["v"][sc][:, e, o:o + 128],
                            w_sb["v"][:, e, :],
                            start=(e == 0), stop=(e == NE - 1))
                    nc.vector.tensor_copy(
                        vsb[:, st, :]
                        .rearrange("p (h u) -> p h u", u=65)[:, :, 0:64],
                        ps[:, :].rearrange("p (h d) -> p h d", d=64))

                # row 0 units get dedicated probs slots (4t+kt) so all four
                # can have scores in flight before any v data exists; later
                # rows share slots kt (Tile serializes WAR per slot)
                def scores_kt(t, j, kt, so):
                    # diagonal tile dd: cols < 128*dd are fully masked ->
                    # compute only cols lo..512 in scores/exp/PV; the band
                    # [lo, lo+128) gets the triangular mask; cols < lo keep
                    # stale slot data that pv_kt never reads
                    dd = kt - 4 * j if causal else -1
                    lo = 128 * dd if dd > 0 else 0
                    ps = sp.tile([128, 1024], f32, tag="s", name=f"ps{t}{j}_{kt}")
                    # A/B heads write the tile's two different PSUM banks
                    # from PE row groups 0/1 -> they run concurrently.
                    for half in (0, 1):
                        po = half * 64
                        nc.tensor.matmul(
                            ps[:, half * 512 + lo:(half + 1) * 512],
                            kh[po:po + 64, t, kt * 128:(kt + 1) * 128],
                            qh[po:po + 64, t, j * 512 + lo:(j + 1) * 512],
                            start=True, stop=True)
                    nc.scalar.activation(
                        probs[:, :, so + kt, lo:512],
                        ps[:, :].rearrange("p (h u) -> p h u", h=2)[:, :, lo:512],
                        EXP)
                    if causal and dd >= 0:
                        for half in (0, 1):
                            nc.vector.tensor_tensor(
                                probs[:, half, so + kt, lo:lo + 128],
                                probs[:, half, so + kt, lo:lo + 128],
                                msk[:, :], MUL)

                def pv_kt(t, j, kt, so, pt, nkt):
                    dd = kt - 4 * j if causal else -1
                    lo = 128 * dd if (dd > 0 and kt > 0) else 0
                    for half in (0, 1):
                        lh = 2 * t + half
                        nc.tensor.matmul(
                            pt[half][:, lo:512],
                            vsb[:, kt, lh * 65:(lh + 1) * 65],
                            probs[:, half, so + kt, lo:512],
                            start=(kt == 0), stop=(kt == nkt - 1))

                def unit_out(t, j, pt):
                    for half in (0, 1):
                        lh = 2 * t + half
                        ost = op.tile([65, 512], f32, tag="ost", name=f"os{half}_{t}{j}")
                        nc.vector.tensor_copy(ost[:, :], pt[half][:, :])
                        nc.sync.dma_start(
                            out=outT_d[lh * 64:(lh + 1) * 64, j * 512:(j + 1) * 512],
                            in_=ost[0:64, :])
                        nc.sync.dma_start(
                            out=l_d[lh:lh + 1, j * 512:(j + 1) * 512],
                            in_=ost[64:65, :])

                def pv_unit(t, j, so):
                    nkt = 4 * (j + 1) if causal else NKT
                    pt = {0: ptA.tile([65, 512], f32, tag="t0", name=f"pt0_{t}{j}"),
                          1: ptB.tile([65, 512], f32, tag="t1", name=f"pt1_{t}{j}")}
                    for kt in range(nkt):
                        pv_kt(t, j, kt, so, pt, nkt)
                    unit_out(t, j, pt)

                def unit(t, j):
                    nkt = 4 * (j + 1) if causal else NKT
                    pt = {0: ptA.tile([65, 512], f32, tag="t0", name=f"pt0_{t}{j}"),
                          1: ptB.tile([65, 512], f32, tag="t1", name=f"pt1_{t}{j}")}
                    # scores run one kt ahead of PV so the PE never waits
                    # head-of-line on the exp of the tile it just produced
                    scores_kt(t, j, 0, 0)
                    for kt in range(1, nkt):
                        scores_kt(t, j, kt, 0)
                        pv_kt(t, j, kt - 1, 0, pt, nkt)
                    pv_kt(t, j, nkt - 1, 0, pt, nkt)
                    unit_out(t, j, pt)

                # pipeline by seq chunk: round j projects chunk j of q/k and
                # v seq-tiles 4j..4j+3; unit (t, j) needs exactly chunks <= j,
                # so row-j units interleave with round-(j+1) projections and
                # ScalarE (exp) saturates from ~7us on
                # ---- schedule: all k projections first, then rows of
                # units in DESCENDING j (3..0) -- the exp-heavy rows overlap
                # the v/q projections and the exp-light row 0 drains the
                # tail.  Each unit's PV stream is zipped into the next
                # unit's scores emission (pv of slot kt right before the
                # next scores write of slot kt), so the PE always has
                # exp-independent work and ScalarE stays saturated.
                pvq = []

                def section(t, j, extra=()):
                    n = 4 * (j + 1) if causal else NKT
                    prev = pvq[:]
                    del pvq[:]
                    pt = {0: ptA.tile([65, 512], f32, tag="t0", name=f"pt0_{t}{j}"),
                          1: ptB.tile([65, 512], f32, tag="t1", name=f"pt1_{t}{j}")}
                    ex = list(extra)
                    for kt in range(n):
                        if kt < len(prev):
                            prev[kt]()
                        scores_kt(t, j, kt, 0)
                        if ex and (kt * len(extra)) // n != ((kt + 1) * len(extra)) // n:
                            ex.pop(0)()
                    for c in prev[n:]:
                        c()
                    for c in ex:
                        c()
                    for kt in range(n):
                        pvq.append(lambda kt=kt: pv_kt(t, j, kt, 0, pt, n))
                    pvq.append(lambda: unit_out(t, j, pt))

                # all v chunks must precede the first pv drain (section
                # (1,3)); later sections take the NEXT row's q chunks as
                # PE filler so every section stays PE-bound (full clock)
                def qc(t, j):
                    return lambda: proj_qk_chunk("q", xh["q"], t, j, qh)

                qfill = {(1, 3): [qc(0, 2)], (2, 3): [qc(1, 2)],
                         (3, 3): [qc(2, 2), qc(3, 2)],
                         (0, 2): [qc(0, 1)], (1, 2): [qc(1, 1)],
                         (2, 2): [qc(2, 1)], (3, 2): [qc(3, 1)],
                         (0, 1): [qc(0, 0)], (1, 1): [qc(1, 0)],
                         (2, 1): [qc(2, 0)], (3, 1): [qc(3, 0)]}
                for t in range(NT):
                    for sc in range(NQC):
                        proj_qk_chunk("k", xh["k"], t, sc, kh)
                    proj_qk_chunk("q", xh["q"], t, 3, qh)
                    flush()
                    ex = ([lambda st=st: proj_v_st(st) for st in range(NKT)]
                          if t == 0 else qfill[(t, 3)])
                    section(t, 3, ex)
                    if t == 0:
                        load_x("q", 2)
                    if t == 2:
                        load_x("q", 1)
                for j in (2, 1, 0):
                    for t in range(NT):
                        if j == 0:
                            flush()
                        section(t, j, qfill.get((t, j), ()))
                        if j == 2 and t == 1:
                            load_x("q", 0)
                for c in pvq:
                    c()
    _built[causal] = nc
    nc.compile()
    return nc


def _prep_core_inputs(c, q, k, v, Wq, bq, Wk, bk, Wv, bv, sin, cos):
    b, hh = c // 2, c % 2
    hs = slice(hh * DH, (hh + 1) * DH)

    perm = np.empty(DH, np.int64)
    for lh in range(HPC):
        base = (hh * HPC + lh) * HD
        perm[lh * HD:lh * HD + HALF] = base + 2 * np.arange(HALF)
        perm[lh * HD + HALF:(lh + 1) * HD] = base + 2 * np.arange(HALF) + 1

    s = 0.125  # 1/sqrt(HD), folded into the q projection
    wq = np.ascontiguousarray((Wq[perm, :] * s).T).astype(BF16)
    wk = np.ascontiguousarray(Wk[perm, :].T).astype(BF16)
    wv = np.ascontiguousarray(Wv[hs, :].T).astype(BF16)

    p32 = np.arange(128) % 32
    cs2 = cos[:, p32].T.astype(BF16)
    sgn = np.where((np.arange(128) // 32) % 2 == 0, -1.0, 1.0).astype(np.float32)
    sn2 = (sin[:, p32] * sgn[None, :]).T.astype(BF16)

    kk = np.arange(128)[:, None]
    msk = (kk <= kk.T).astype(BF16)  # [128,128] lower-tri in q-major sense
    psw = np.zeros((128, 128), np.float32)
    psw[np.arange(128), np.arange(128) ^ 32] = 1.0

    return {
        "xq": np.ascontiguousarray(q[b].T).astype(BF16),
        "xk": np.ascontiguousarray(k[b].T).astype(BF16),
        "xv": np.ascontiguousarray(v[b].T).astype(BF16),
        "wq": wq, "wk": wk, "wv": wv,
        "bqc": np.ascontiguousarray((bq[perm] * s).reshape(NT, 128).T, np.float32),
        "bkc": np.ascontiguousarray(bk[perm].reshape(NT, 128).T, np.float32),
        "cs2": cs2, "sn2": sn2, "msk": msk, "psw": psw.astype(BF16),
    }


def prep_in_maps(q, k, v, Wq, bq, Wk, bk, Wv, bv, sin, cos):
    args = [np.asarray(a, np.float32) for a in (q, k, v, Wq, bq, Wk, bk, Wv, bv, sin, cos)]
    maps = [_prep_core_inputs(c, *args) for c in range(8)]
    return maps, args[8]  # bv needed on host in assemble()


def assemble(results, bv):
    out = np.empty((BATCH, SEQ, EMB), np.float32)
    for c in range(8):
        b, hh = c // 2, c % 2
        outT = np.asarray(results[c]["outT"], np.float32)
        l = np.asarray(results[c]["lsum"], np.float32)
        a = outT.reshape(HPC, HD, SEQ) / l[:, None, :]
        out[b, :, hh * DH:(hh + 1) * DH] = a.reshape(DH, SEQ).T \
            + bv[hh * DH:(hh + 1) * DH][None, :]
    return out


def run(in_maps, causal=True, trace=False, **kw):
    _install_ntff_shim()
    from concourse.bass_utils import run_bass_kernel_spmd
    nc = build(causal)
    return run_bass_kernel_spmd(nc, in_maps, core_ids=list(range(8)), trace=trace, **kw)


def kernel(q, k, v, Wq, bq, Wk, bk, Wv, bv, sin, cos, mask):
    in_maps, bv_f = prep_in_maps(q, k, v, Wq, bq, Wk, bk, Wv, bv, sin, cos)
    r = run(in_maps, causal=bool(mask))
    return assemble(r.results, bv_f)

